# revision 41
# baseline (speedup 1.0000x reference)
"""Trainium2 Bass kernel for a 2D correlation layer.

out[b, dx*41+dy, h, w] = sum_c x[b,c,h,w] * xpad[b,c,h+dx,w+dy]
with x of shape (4, 256, 64, 128), 41x41 displacements (max_disp 20).

Strategy (8 NeuronCores, SPMD):
  - Shard by (batch b, h-half): core = b*2 + half, each core computes
    out[b, :, H0:H0+32, :] for H0 = 32*half.
  - Host pre-pads each core's input to a slab [256, 72, 168] (zeros).
  - Per output row h: 4 column-tiled matmuls (32 pixels each) with
    per-tile shifted rhs windows contract over C=256 into PSUM
    S[pixel(w), (dx, u=v+dy)]  (v = w mod 32).
  - The per-pixel displacement shear (dy = u - v) is affine in DMA
    coordinates: dump S to a padded DRAM intermediate T[p, dx, dyp]
    with per-8-partition-block windows, then reload displacement-major
    Z[w, (dx, dy)].
  - PE transposes flip Z into Zt[d, w] so the final store writes the
    output with contiguous 512B w-runs.
"""

import ml_dtypes
import numpy as np

import concourse.bass as bass
import concourse.mybir as mybir
import concourse.tile as tile
from concourse.vector_clock import ScopedClock

F32 = mybir.dt.float32
F32R = mybir.dt.float32r
BF16 = mybir.dt.bfloat16

# ---------------------------------------------------------------------------
# Toolchain patches: this walrus build allows at most ONE sync-wait per
# instruction. (a) split the final TileContext drain's waits; (b) split any
# other multi-wait instruction at the BIR-JSON level before compilation.
# ---------------------------------------------------------------------------


def _patched_drain_and_barrier(self, tick_clock, wait_clock):
    drain_inst = self.nc.sync.drain()
    wait_clock.add_sem_waits(
        drain_inst.ins, ScopedClock({None: tick_clock.global_clock})
    )
    si = drain_inst.ins.sync_info
    if si is not None and len(si.on_wait) > 1:
        waits = list(si.on_wait)
        drain_inst.ins.sync_info = mybir.SyncInfo(
            on_wait=[waits[0]], on_update=list(si.on_update)
        )
        for w in waits[1:]:
            nop = self.nc.sync.nop(nofuse=True, hint="split_drain_wait")
            nop.ins.sync_info = mybir.SyncInfo(on_wait=[w], on_update=[])

    self.nc.all_engine_barrier()
    assert self.sems is not None
    popped = self.nc._tile_sem_poison_stack.pop()
    assert popped is self._sem_poison
    self.nc.clear_and_free_semaphores(list(self.sems.allocated().values()))
    self.nc.all_engine_barrier()


tile.TileContext._drain_and_barrier = _patched_drain_and_barrier

import orjson as _orjson
import concourse.bass_utils as _bass_utils


def _split_multi_waits_json(bir_json: bytes) -> bytes:
    bir = _orjson.loads(bir_json)
    counter = [0]
    changed = False
    for fn in bir.get("functions", []):
        for bb in fn.get("blocks", []) or []:
            insts = bb.get("instructions")
            if insts is None:
                continue
            new_insts = []
            for ins in insts:
                si = ins.get("sync_info")
                if si and len(si.get("on_wait") or []) > 1:
                    waits = si["on_wait"]
                    for w in waits[:-1]:
                        counter[0] += 1
                        new_insts.append({
                            "name": f"I-wsplit-{counter[0]}",
                            "opcode": "NoOp",
                            "engine": ins["engine"],
                            "ins": [],
                            "outs": [],
                            "sync_info": {"on_wait": [w], "on_update": []},
                        })
                    si["on_wait"] = [waits[-1]]
                    changed = True
                new_insts.append(ins)
            bb["instructions"] = new_insts
    if not changed:
        return bir_json
    return _orjson.dumps(bir)


_orig_compile_bir_kernel = _bass_utils.compile_bir_kernel


def _patched_compile_bir_kernel(bir_json, tmpdir, neff_name="file.neff"):
    return _orig_compile_bir_kernel(
        _split_multi_waits_json(bir_json), tmpdir, neff_name
    )


if getattr(_bass_utils.compile_bir_kernel, "__name__", "") != "_patched_compile_bir_kernel":
    _bass_utils.compile_bir_kernel = _patched_compile_bir_kernel
    try:
        import concourse.bass2jax as _bass2jax

        _bass2jax.compile_bir_kernel = _patched_compile_bir_kernel
    except Exception:
        pass

# ---------------------------------------------------------------------------
# Problem constants (hardcoded; kernel.py must be self-contained)
# ---------------------------------------------------------------------------
B, C, H, W = 4, 256, 64, 128
MD = 20
ND = 2 * MD + 1            # 41 displacements per axis
D2 = ND * ND               # 1681
HH = H // 2                # 32 h rows per core
SLABH = HH + 2 * MD        # 72
SLABW = W + 2 * MD         # 168
SROW = 2 * SLABH * SLABW   # slab free-dim row length per partition (24192)
CT = 16                    # col-tile width (pixels)
UW = CT + 2 * MD           # 56  u-window per col-tile
AH = 128 // CT             # 8 h-rows per weight rect (M = AH*CT = 128)
NJ = W // CT               # 8 w-tiles
SH = AH + 2 * MD           # 48  s-rows per rect
SR = SH * UW               # 2688 columns of S per rect per partition
DYP = UW                   # dyp stride == u-run width -> dump dst folds
TROW = ND * DYP + CT - 1   # +15 tail so spill stays off next partition
TST = 128 * TROW           # T elements per output row
NFULL = D2 // 128          # 13 full transpose chunks
NPART = D2 - NFULL * 128   # 17 leftover displacement channels
# s-row chunking for PSUM banks (N <= 512 fp32 per matmul)
S_CHUNKS = [(0, 9), (9, 9), (18, 9), (27, 9), (36, 9), (45, 3)]


def _build_nc():
    nc = bass.Bass()
    xpad = nc.declare_dram_parameter("xpad", [C, SLABH, SLABW], BF16, isOutput=False)
    out = nc.declare_dram_parameter("out", [D2, HH, W], F32, isOutput=True)
    T = nc.dram_tensor("T", [HH, 128, TROW], BF16)

    with tile.TileContext(nc) as tc:
        with (
            tc.tile_pool(name="slab", bufs=1) as slab_pool,
            tc.tile_pool(name="ident", bufs=1) as ident_pool,
            tc.tile_pool(name="ssb", bufs=2) as spool,
            tc.tile_pool(name="wt", bufs=3) as wpool,
            tc.tile_pool(name="z", bufs=4) as zpool,
            tc.tile_pool(name="zt", bufs=4) as ztpool,
            tc.tile_pool(name="pchunk", bufs=5, space=bass.MemorySpace.PSUM) as pchunk,
            tc.tile_pool(name="ptr", bufs=3, space=bass.MemorySpace.PSUM) as ptr,
        ):
            # persistent padded input slab: [c-partition, (chalf, hh, ww)]
            slab = slab_pool.tile([128, 2, SLABH, SLABW], BF16)
            # xpad[c, hh, ww] -> slab[c % 128, c // 128, hh, ww]
            nc.sync.dma_start(
                slab[:],
                xpad[:].rearrange("(ch p) hh ww -> p ch hh ww", ch=2),
            )

            # identity for PE transpose (bf16 to match the sheared S dtype)
            ident_i = ident_pool.tile([128, 128], mybir.dt.int32)
            nc.gpsimd.iota(ident_i[:], pattern=[[1, 128]], base=0,
                           channel_multiplier=-1)
            ident = ident_pool.tile([128, 128], BF16)
            nc.vector.tensor_scalar(ident[:], ident_i[:], 0, None,
                                    mybir.AluOpType.is_equal)

            slab_t = slab[:].tensor
            assert isinstance(slab[:].offset, int) and slab[:].offset == 0

            import os
            n_st = int(os.environ.get("KERNEL_ST_LIMIT", HH))
            n_hq = max(1, n_st // AH)

            def matmul_phase(hq):
                # ---- rect matmuls: S[(rh, wl), (j, s, u)], M = 128 ----
                # all NJ w-tiles share one big tile so the shear dump is a
                # single 3-dim DMA (j = free-dim stride SR).
                s_big = spool.tile([128, NJ * SR], BF16)
                all_copies = []
                for j in range(NJ):
                    # contiguous weight gather: AH h-rows x CT w pixels per ch
                    wts = wpool.tile([128, 2, 128], BF16)
                    for ch in range(2):
                        nc.vector.tensor_copy(
                            wts[:, ch, :].rearrange("p (a c) -> p a c", a=AH),
                            bass.AP(
                                slab_t,
                                ch * (SLABH * SLABW) + (AH * hq + MD) * SLABW
                                + CT * j + MD,
                                [[SROW, 128], [SLABW, AH], [1, CT]],
                            ),
                        )
                    for s0, ns in S_CHUNKS:
                        ps = pchunk.tile([128, 9 * UW], F32, tag="ps")
                        n = ns * UW
                        for ch in range(2):
                            rhs = bass.AP(
                                slab_t,
                                ch * (SLABH * SLABW) + (AH * hq + s0) * SLABW
                                + CT * j,
                                [[SROW, 128], [SLABW, ns], [1, UW]],
                            )
                            nc.tensor.matmul(
                                ps[:, 0:n], wts[:, ch, :], rhs,
                                start=(ch == 0), stop=(ch == 1),
                            )
                        all_copies.append(
                            nc.vector.tensor_copy(
                                s_big[:, j * SR + s0 * UW:j * SR + s0 * UW + n],
                                ps[:, 0:n],
                            )
                        )
                return s_big, all_copies

            def row_phase(hq, s_big, all_copies):
                s_t2 = s_big[:].tensor
                s_off2 = s_big[:].offset
                for rh in range(AH):
                    st = AH * hq + rh
                    # ---- shear hop 1: one DMA. src partition CT*rh + wl,
                    # j via free-dim stride SR; dst T[st, CT*j+wl, ...] with
                    # per-partition -1 shift so dyp = dy + CT - 1 uniformly.
                    src = bass.AP(
                        s_t2,
                        s_off2 + CT * rh * (NJ * SR) + rh * UW,
                        [[NJ * SR, CT], [SR, NJ], [1, ND * UW]],
                    )
                    dst = bass.AP(
                        T,
                        st * TST + CT - 1,
                        [[TROW - 1, CT], [CT * TROW, NJ], [1, ND * UW]],
                    )
                    d = nc.sync.dma_start(dst, src)
                    for cp in all_copies:
                        tile.add_dep_helper(d.ins, cp.ins, reason="S->shear")

                    # ---- shear hop 2: packed displacement-major gather ----
                    z = zpool.tile([128, D2], BF16)
                    rl = nc.sync.dma_start(
                        z[:],
                        bass.AP(T, st * TST + CT - 1,
                                [[TROW, 128], [DYP, ND], [1, ND]]),
                    )
                    tile.add_dep_helper(rl.ins, d.ins, reason="dump->reload")

                    # ---- transpose to Zt[d, (chunk, w)] and store ----
                    zt = ztpool.tile([128, (NFULL + 1) * 128], F32)
                    for cix in range(NFULL + 1):
                        ncols = 128 if cix < NFULL else NPART
                        tp = ptr.tile([128, 128], BF16, tag="tp")
                        tr = nc.tensor.transpose(
                            tp[0:ncols, :], z[:, 128 * cix:128 * cix + ncols],
                            ident[:],
                        )
                        tile.add_dep_helper(tr.ins, rl.ins, reason="shear->tr")
                        nc.scalar.copy(
                            zt[0:ncols, 128 * cix:128 * cix + 128],
                            tp[0:ncols, :],
                        )

                    zt_t = zt[:].tensor
                    zt_off = zt[:].offset
                    # full chunks: d = 128*c + dd
                    nc.gpsimd.dma_start(
                        bass.AP(out, st * W,
                                [[HH * W, 128], [128 * HH * W, NFULL], [1, W]]),
                        bass.AP(zt_t, zt_off,
                                [[(NFULL + 1) * 128, 128], [128, NFULL], [1, W]]),
                    )
                    nc.gpsimd.dma_start(
                        bass.AP(out, NFULL * 128 * HH * W + st * W,
                                [[HH * W, NPART], [1, W]]),
                        bass.AP(zt_t, zt_off + NFULL * 128,
                                [[(NFULL + 1) * 128, NPART], [1, W]]),
                    )

            # software pipeline: hq's matmuls overlap hq-1's shear/transpose
            prev = None
            for hq in range(n_hq):
                state = matmul_phase(hq)
                if prev is not None:
                    row_phase(prev[0], prev[1], prev[2])
                prev = (hq, state[0], state[1])
            row_phase(prev[0], prev[1], prev[2])
    return nc


_CACHE = {}


def _get_nc():
    if "nc" not in _CACHE:
        _CACHE["nc"] = _build_nc()
    return _CACHE["nc"]


def kernel(x_1: np.ndarray, _trace: bool = False) -> np.ndarray:
    from concourse.bass_utils import run_bass_kernel_spmd

    x_1 = np.ascontiguousarray(x_1, dtype=np.float32)
    assert x_1.shape == (B, C, H, W)
    nc = _get_nc()

    in_maps = []
    for core in range(8):
        b, half = core // 2, core % 2
        H0 = HH * half
        slab = np.zeros((C, SLABH, SLABW), ml_dtypes.bfloat16)
        h_lo = max(0, H0 - MD)
        h_hi = min(H, H0 + HH + MD)
        slab[:, (h_lo - H0 + MD):(h_hi - H0 + MD), MD:MD + W] = x_1[
            b, :, h_lo:h_hi, :
        ].astype(ml_dtypes.bfloat16)
        in_maps.append({"xpad": slab})

    res = run_bass_kernel_spmd(nc, in_maps, core_ids=list(range(8)), trace=_trace)
    _CACHE["last_results"] = res
    out = np.empty((B, D2, H, W), np.float32)
    for core in range(8):
        b, half = core // 2, core % 2
        out[b, :, HH * half:HH * (half + 1), :] = res.results[core]["out"]
    return out


if __name__ == "__main__":
    x = np.random.randn(B, C, H, W).astype(np.float32)
    y = kernel(x)
    print("kernel output shape:", y.shape)



# revision 44
# speedup vs baseline: 1.0464x; 1.0464x over previous
"""Trainium2 Bass kernel for a 2D correlation layer.

out[b, dx*41+dy, h, w] = sum_c x[b,c,h,w] * xpad[b,c,h+dx,w+dy]
with x of shape (4, 256, 64, 128), 41x41 displacements (max_disp 20).

Strategy (8 NeuronCores, SPMD):
  - Shard by (batch b, h-half): core = b*2 + half, each core computes
    out[b, :, H0:H0+32, :] for H0 = 32*half.
  - Host pre-pads each core's input to a slab [256, 72, 168] (zeros).
  - Per output row h: 4 column-tiled matmuls (32 pixels each) with
    per-tile shifted rhs windows contract over C=256 into PSUM
    S[pixel(w), (dx, u=v+dy)]  (v = w mod 32).
  - The per-pixel displacement shear (dy = u - v) is affine in DMA
    coordinates: dump S to a padded DRAM intermediate T[p, dx, dyp]
    with per-8-partition-block windows, then reload displacement-major
    Z[w, (dx, dy)].
  - PE transposes flip Z into Zt[d, w] so the final store writes the
    output with contiguous 512B w-runs.
"""

import ml_dtypes
import numpy as np

import concourse.bass as bass
import concourse.mybir as mybir
import concourse.tile as tile
from concourse.vector_clock import ScopedClock

F32 = mybir.dt.float32
F32R = mybir.dt.float32r
BF16 = mybir.dt.bfloat16

# ---------------------------------------------------------------------------
# Toolchain patches: this walrus build allows at most ONE sync-wait per
# instruction. (a) split the final TileContext drain's waits; (b) split any
# other multi-wait instruction at the BIR-JSON level before compilation.
# ---------------------------------------------------------------------------


def _patched_drain_and_barrier(self, tick_clock, wait_clock):
    drain_inst = self.nc.sync.drain()
    wait_clock.add_sem_waits(
        drain_inst.ins, ScopedClock({None: tick_clock.global_clock})
    )
    si = drain_inst.ins.sync_info
    if si is not None and len(si.on_wait) > 1:
        waits = list(si.on_wait)
        drain_inst.ins.sync_info = mybir.SyncInfo(
            on_wait=[waits[0]], on_update=list(si.on_update)
        )
        for w in waits[1:]:
            nop = self.nc.sync.nop(nofuse=True, hint="split_drain_wait")
            nop.ins.sync_info = mybir.SyncInfo(on_wait=[w], on_update=[])

    self.nc.all_engine_barrier()
    assert self.sems is not None
    popped = self.nc._tile_sem_poison_stack.pop()
    assert popped is self._sem_poison
    self.nc.clear_and_free_semaphores(list(self.sems.allocated().values()))
    self.nc.all_engine_barrier()


tile.TileContext._drain_and_barrier = _patched_drain_and_barrier

import orjson as _orjson
import concourse.bass_utils as _bass_utils


def _split_multi_waits_json(bir_json: bytes) -> bytes:
    bir = _orjson.loads(bir_json)
    counter = [0]
    changed = False
    for fn in bir.get("functions", []):
        for bb in fn.get("blocks", []) or []:
            insts = bb.get("instructions")
            if insts is None:
                continue
            new_insts = []
            for ins in insts:
                si = ins.get("sync_info")
                if si and len(si.get("on_wait") or []) > 1:
                    waits = si["on_wait"]
                    for w in waits[:-1]:
                        counter[0] += 1
                        new_insts.append({
                            "name": f"I-wsplit-{counter[0]}",
                            "opcode": "NoOp",
                            "engine": ins["engine"],
                            "ins": [],
                            "outs": [],
                            "sync_info": {"on_wait": [w], "on_update": []},
                        })
                    si["on_wait"] = [waits[-1]]
                    changed = True
                new_insts.append(ins)
            bb["instructions"] = new_insts
    if not changed:
        return bir_json
    return _orjson.dumps(bir)


_orig_compile_bir_kernel = _bass_utils.compile_bir_kernel


def _patched_compile_bir_kernel(bir_json, tmpdir, neff_name="file.neff"):
    return _orig_compile_bir_kernel(
        _split_multi_waits_json(bir_json), tmpdir, neff_name
    )


if getattr(_bass_utils.compile_bir_kernel, "__name__", "") != "_patched_compile_bir_kernel":
    _bass_utils.compile_bir_kernel = _patched_compile_bir_kernel
    try:
        import concourse.bass2jax as _bass2jax

        _bass2jax.compile_bir_kernel = _patched_compile_bir_kernel
    except Exception:
        pass

# ---------------------------------------------------------------------------
# Problem constants (hardcoded; kernel.py must be self-contained)
# ---------------------------------------------------------------------------
B, C, H, W = 4, 256, 64, 128
MD = 20
ND = 2 * MD + 1            # 41 displacements per axis
D2 = ND * ND               # 1681
HH = H // 2                # 32 h rows per core
SLABH = HH + 2 * MD        # 72
SLABW = W + 2 * MD         # 168
SROW = 2 * SLABH * SLABW   # slab free-dim row length per partition (24192)
CT = 16                    # col-tile width (pixels)
UW = CT + 2 * MD           # 56  u-window per col-tile
AH = 128 // CT             # 8 h-rows per weight rect (M = AH*CT = 128)
NJ = W // CT               # 8 w-tiles
SH = AH + 2 * MD           # 48  s-rows per rect
SR = SH * UW               # 2688 columns of S per rect per partition
DYP = UW                   # dyp stride == u-run width -> dump dst folds
TROW = ND * DYP + CT - 1   # +15 tail so spill stays off next partition
TST = 128 * TROW           # T elements per output row
NFULL = D2 // 128          # 13 full transpose chunks
NPART = D2 - NFULL * 128   # 17 leftover displacement channels
# s-row chunking for PSUM banks (N <= 512 fp32 per matmul)
S_CHUNKS = [(0, 9), (9, 9), (18, 9), (27, 9), (36, 9), (45, 3)]


def _build_nc():
    nc = bass.Bass()
    xpad = nc.declare_dram_parameter("xpad", [C, SLABH, SLABW], BF16, isOutput=False)
    out = nc.declare_dram_parameter("out", [D2, HH, W], F32, isOutput=True)
    T = nc.dram_tensor("T", [HH, 128, TROW], BF16)

    with tile.TileContext(nc) as tc:
        with (
            tc.tile_pool(name="slab", bufs=1) as slab_pool,
            tc.tile_pool(name="ident", bufs=1) as ident_pool,
            tc.tile_pool(name="ssb", bufs=2) as spool,
            tc.tile_pool(name="wt", bufs=3) as wpool,
            tc.tile_pool(name="z", bufs=4) as zpool,
            tc.tile_pool(name="zt", bufs=4) as ztpool,
            tc.tile_pool(name="pchunk", bufs=5, space=bass.MemorySpace.PSUM) as pchunk,
            tc.tile_pool(name="ptr", bufs=3, space=bass.MemorySpace.PSUM) as ptr,
        ):
            # persistent padded input slab: [c-partition, (chalf, hh, ww)]
            slab = slab_pool.tile([128, 2, SLABH, SLABW], BF16)
            # xpad[c, hh, ww] -> slab[c % 128, c // 128, hh, ww]
            nc.sync.dma_start(
                slab[:],
                xpad[:].rearrange("(ch p) hh ww -> p ch hh ww", ch=2),
            )

            # identity for PE transpose (bf16 to match the sheared S dtype)
            ident_i = ident_pool.tile([128, 128], mybir.dt.int32)
            nc.gpsimd.iota(ident_i[:], pattern=[[1, 128]], base=0,
                           channel_multiplier=-1)
            ident = ident_pool.tile([128, 128], BF16)
            nc.vector.tensor_scalar(ident[:], ident_i[:], 0, None,
                                    mybir.AluOpType.is_equal)

            slab_t = slab[:].tensor
            assert isinstance(slab[:].offset, int) and slab[:].offset == 0

            import os
            n_st = int(os.environ.get("KERNEL_ST_LIMIT", HH))
            n_hq = max(1, n_st // AH)

            def matmul_phase(hq):
                # ---- rect matmuls: S[(rh, wl), (j, s, u)], M = 128 ----
                # all NJ w-tiles share one big tile so the shear dump is a
                # single 3-dim DMA (j = free-dim stride SR).
                s_big = spool.tile([128, NJ * SR], BF16)
                all_copies = []
                for j in range(NJ):
                    # contiguous weight gather: AH h-rows x CT w pixels per ch
                    wts = wpool.tile([128, 2, 128], BF16)
                    for ch in range(2):
                        nc.vector.tensor_copy(
                            wts[:, ch, :].rearrange("p (a c) -> p a c", a=AH),
                            bass.AP(
                                slab_t,
                                ch * (SLABH * SLABW) + (AH * hq + MD) * SLABW
                                + CT * j + MD,
                                [[SROW, 128], [SLABW, AH], [1, CT]],
                            ),
                        )
                    for s0, ns in S_CHUNKS:
                        ps = pchunk.tile([128, 9 * UW], F32, tag="ps")
                        n = ns * UW
                        for ch in range(2):
                            rhs = bass.AP(
                                slab_t,
                                ch * (SLABH * SLABW) + (AH * hq + s0) * SLABW
                                + CT * j,
                                [[SROW, 128], [SLABW, ns], [1, UW]],
                            )
                            nc.tensor.matmul(
                                ps[:, 0:n], wts[:, ch, :], rhs,
                                start=(ch == 0), stop=(ch == 1),
                            )
                        all_copies.append(
                            nc.vector.tensor_copy(
                                s_big[:, j * SR + s0 * UW:j * SR + s0 * UW + n],
                                ps[:, 0:n],
                            )
                        )
                return s_big, all_copies

            def row_phase(hq, s_big, all_copies):
                s_t2 = s_big[:].tensor
                s_off2 = s_big[:].offset
                for rh in range(AH):
                    st = AH * hq + rh
                    # ---- shear hop 1: one DMA. src partition CT*rh + wl,
                    # j via free-dim stride SR; dst T[st, CT*j+wl, ...] with
                    # per-partition -1 shift so dyp = dy + CT - 1 uniformly.
                    src = bass.AP(
                        s_t2,
                        s_off2 + CT * rh * (NJ * SR) + rh * UW,
                        [[NJ * SR, CT], [SR, NJ], [1, ND * UW]],
                    )
                    dst = bass.AP(
                        T,
                        st * TST + CT - 1,
                        [[TROW - 1, CT], [CT * TROW, NJ], [1, ND * UW]],
                    )
                    d = nc.sync.dma_start(dst, src)
                    for cp in all_copies:
                        tile.add_dep_helper(d.ins, cp.ins, reason="S->shear")

                    # ---- shear hop 2: packed displacement-major gather ----
                    z = zpool.tile([128, D2], BF16)
                    rl = nc.sync.dma_start(
                        z[:],
                        bass.AP(T, st * TST + CT - 1,
                                [[TROW, 128], [DYP, ND], [1, ND]]),
                    )
                    tile.add_dep_helper(rl.ins, d.ins, reason="dump->reload")

                    # ---- transpose to Zt[d, (chunk, w)] and store ----
                    zt = ztpool.tile([128, (NFULL + 1) * 128], F32)
                    for cix in range(NFULL + 1):
                        ncols = 128 if cix < NFULL else NPART
                        tp = ptr.tile([128, 128], BF16, tag="tp")
                        tr = nc.tensor.transpose(
                            tp[0:ncols, :], z[:, 128 * cix:128 * cix + ncols],
                            ident[:],
                        )
                        tile.add_dep_helper(tr.ins, rl.ins, reason="shear->tr")
                        nc.scalar.copy(
                            zt[0:ncols, 128 * cix:128 * cix + 128],
                            tp[0:ncols, :],
                        )

                    zt_t = zt[:].tensor
                    zt_off = zt[:].offset
                    # full chunks: d = 128*c + dd
                    nc.gpsimd.dma_start(
                        bass.AP(out, st * W,
                                [[HH * W, 128], [128 * HH * W, NFULL], [1, W]]),
                        bass.AP(zt_t, zt_off,
                                [[(NFULL + 1) * 128, 128], [128, NFULL], [1, W]]),
                    )
                    nc.gpsimd.dma_start(
                        bass.AP(out, NFULL * 128 * HH * W + st * W,
                                [[HH * W, NPART], [1, W]]),
                        bass.AP(zt_t, zt_off + NFULL * 128,
                                [[(NFULL + 1) * 128, NPART], [1, W]]),
                    )

            # software pipeline: hq's matmuls overlap hq-1's shear/transpose
            prev = None
            for hq in range(n_hq):
                state = matmul_phase(hq)
                if prev is not None:
                    row_phase(prev[0], prev[1], prev[2])
                prev = (hq, state[0], state[1])
            row_phase(prev[0], prev[1], prev[2])
    return nc


_CACHE = {}


def _get_nc():
    if "nc" not in _CACHE:
        _CACHE["nc"] = _build_nc()
    return _CACHE["nc"]


def kernel(x_1: np.ndarray, _trace: bool = False) -> np.ndarray:
    from concourse.bass_utils import run_bass_kernel_spmd

    x_1 = np.ascontiguousarray(x_1, dtype=np.float32)
    assert x_1.shape == (B, C, H, W)
    nc = _get_nc()

    in_maps = []
    for core in range(8):
        b, half = core // 2, core % 2
        H0 = HH * half
        slab = np.zeros((C, SLABH, SLABW), ml_dtypes.bfloat16)
        h_lo = max(0, H0 - MD)
        h_hi = min(H, H0 + HH + MD)
        slab[:, (h_lo - H0 + MD):(h_hi - H0 + MD), MD:MD + W] = x_1[
            b, :, h_lo:h_hi, :
        ].astype(ml_dtypes.bfloat16)
        in_maps.append({"xpad": slab})

    res = run_bass_kernel_spmd(nc, in_maps, core_ids=list(range(8)), trace=_trace)
    _CACHE["last_results"] = res
    out = np.empty((B, D2, H, W), np.float32)
    for core in range(8):
        b, half = core // 2, core % 2
        out[b, :, HH * half:HH * (half + 1), :] = res.results[core]["out"]
    return out


if __name__ == "__main__":
    x = np.random.randn(B, C, H, W).astype(np.float32)
    y = kernel(x)
    print("kernel output shape:", y.shape)

